# revision 25
# baseline (speedup 1.0000x reference)
"""Trainium2 Bass kernel: multi-table embedding gather (pooling=NONE).

Reference computation (hardcoded shapes):
    indices: [F=4, BL=204800] int   (values in [0, V))
    tables:  [F=4, V=1e6, D=64] f32
    out[f]   = tables[PERM[f]][indices[PERM[f]]]   -> [4, 204800, 64] f32
    PERM = [2, 0, 3, 1]

Strategy (model/table-parallel, per the sharding hint):
  * Fold the table permutation into global row ids g = PERM[f]*V + idx over a
    flat [4M, 64] table.
  * Shard the flat table row-wise across the 8 cores (500,000 rows each).
    The host routes every lookup to the core owning its row, bucketing each
    core's lookups by 32,768-row window so the on-core gather can use the
    high-throughput int16 `dma_gather` SWDGE instruction (0.34 ns/descriptor
    vs ~1 us/instruction for generic indirect DMA, which HW only supports at
    128 rows per instruction).
  * Each core gathers its (padded) buckets window-by-window into SBUF and
    streams them to a contiguous staging output with large HWDGE DMAs.
  * The host applies the recorded inverse permutation to scatter staged rows
    into the final [4, 204800, 64] output (host-side unshard).
"""

import sys

import numpy as np

for _p in ("/opt/trn_rl_repo",):
    if _p not in sys.path:
        sys.path.insert(0, _p)

F = 4
V = 1_000_000
D = 64
BL = 204_800
PERM = (2, 0, 3, 1)

N_CORES = 8
P = 128
ROWS_TOTAL = F * BL                   # 819,200 lookups
SHARD = F * V // N_CORES              # 500,000 table rows per core
WIN = 32_768                          # int16-addressable window
N_FULL_WIN = SHARD // WIN             # 15 full windows
LAST_WIN_ROWS = SHARD - N_FULL_WIN * WIN  # 8,480
N_WIN = N_FULL_WIN + 1                # 16 windows per core

# Per-window bucket capacity (static padding; lookups are uniform so bucket
# sizes concentrate tightly: full-window mean 6711 sigma 82, last-window mean
# 1737 sigma 42).
PAD_FULL = 7_168                      # 56 * 128
PAD_LAST = 2_048                      # 16 * 128
PADS = [PAD_FULL] * N_FULL_WIN + [PAD_LAST]
COLS = [p // P for p in PADS]         # dst free-dim blocks per window
STAGE_ROWS = sum(PADS)                # 109,568 staged rows per core
IDX_COLS = sum(p // 16 for p in PADS)  # int16 idx columns: 6,848
WIN_ROWS = [WIN] * N_FULL_WIN + [LAST_WIN_ROWS]

NBUF = 7                # window dst tiles in flight (14KB/partition each)
GRANULE = 1024          # idxs per dma_gather (single-packet limit: 64 desc/engine)
GCOLS = GRANULE // P    # 8 dst free-dim blocks per sub-gather
N_SWDGE_QUEUES = 4
DMA_SCRATCH = 65536
NSUB = sum(p // GRANULE for p in PADS)   # 107 sub-gathers per core


def build_nc():
    """Per-core SPMD program: 16 dma_gather windows + contiguous writebacks."""
    import concourse.bacc as bacc
    import concourse.mybir as mybir
    import concourse.tile as tile

    nc = bacc.Bacc(
        None,
        num_swdge_queues=N_SWDGE_QUEUES,
        dynamic_dma_scratch_size=DMA_SCRATCH,
    )
    tabs = [
        nc.declare_dram_parameter(
            f"tab{w}", [WIN_ROWS[w], D], mybir.dt.float32, isOutput=False
        )
        for w in range(N_WIN)
    ]
    idx_in = nc.declare_dram_parameter(
        "idx", [P, IDX_COLS], mybir.dt.int16, isOutput=False
    )
    cnt_in = nc.declare_dram_parameter(
        "cnt", [1, NSUB + N_WIN], mybir.dt.int32, isOutput=False
    )
    out = nc.declare_dram_parameter(
        "out", [STAGE_ROWS, D], mybir.dt.float32, isOutput=True
    )

    idx_off = np.cumsum([0] + [p // 16 for p in PADS]).tolist()
    stage_off = np.cumsum([0] + PADS).tolist()

    g_idx = 0
    with tile.TileContext(nc) as tc:
        with (
            tc.tile_pool(name="idxp", bufs=1) as idxpool,
            tc.tile_pool(name="data", bufs=NBUF) as pool,
        ):
            idx_tile = idxpool.tile([P, IDX_COLS], mybir.dt.int16)
            cnt_tile = idxpool.tile([1, NSUB + N_WIN], mybir.dt.int32, tag="cnt")
            nc.sync.dma_start(out=cnt_tile[:], in_=cnt_in[:])
            # Tiny dummy gather up front so Bacc's auto library reload lands
            # at the top of the Pool stream, overlapping the preamble instead
            # of gating the first real gather.
            dummy_idx = idxpool.tile([P, 1], mybir.dt.int16, tag="dmyi")
            dummy_dst = idxpool.tile([P, 4 * D], mybir.dt.float32, tag="dmyd")
            nc.vector.memset(dummy_idx[:], 0)
            # one per queue: keeps Tile's DMASW lane<->queue phase aligned
            # for the real gather stream (lane k stays locked to queue k%4)
            for q in range(N_SWDGE_QUEUES):
                nc.gpsimd.dma_gather(
                    dummy_dst[:, q * D : (q + 1) * D].rearrange(
                        "p (c d) -> p c d", d=D
                    ),
                    tabs[0][:],
                    dummy_idx[:],
                    16,
                    16,
                    D,
                    single_packet=True,
                    queue_num=q,
                )
            # split the idx load per window so window 0 can start gathering
            # without waiting for the full 1.75MB index transfer
            for w in range(N_WIN):
                nc.sync.dma_start(
                    out=idx_tile[:, idx_off[w] : idx_off[w + 1]],
                    in_=idx_in[:, idx_off[w] : idx_off[w + 1]],
                )
            for w in range(N_WIN):
                # Split each window into 1024-idx single-packet gathers:
                # >64 descriptors/engine exceeds the packet limit (device
                # crash); four SWDGE queues keep ~4 packets in flight per
                # SDMA engine (latency-bound random 256B reads). All
                # sub-gathers of a window land in one tile (disjoint slices)
                # so the writeback is a single 1.8MB HWDGE DMA.
                data = pool.tile([P, COLS[w] * D], mybir.dt.float32, tag="data")
                for s in range(PADS[w] // GRANULE):
                    c0 = idx_off[w] + s * (GRANULE // 16)
                    f0 = s * GRANULE // P * D
                    # Static count: pad slots gather window row 0 (cheap
                    # row-buffer hits); runtime count-skip registers measured
                    # net-slower (reg deps + losing duplicate-row locality).
                    nc.gpsimd.dma_gather(
                        data[:, f0 : f0 + GCOLS * D].rearrange(
                            "p (c d) -> p c d", d=D
                        ),
                        tabs[w][:],
                        idx_tile[:, c0 : c0 + GRANULE // 16],
                        GRANULE,
                        GRANULE,
                        D,
                        single_packet=True,
                        queue_num=g_idx % N_SWDGE_QUEUES,
                    )
                    g_idx += 1
                win_ap = out[stage_off[w] : stage_off[w + 1], :].rearrange(
                    "(p c) d -> p (c d)", p=P
                )
                nc.sync.dma_start(out=win_ap[:], in_=data[:])
    nc.compile()
    return nc


def route(indices):
    """Host-side routing: global ids -> per-core window buckets.

    Returns (idx_inputs [N_CORES, P, IDX_COLS] int16,
             dst_rows   [ROWS_TOTAL] original flat output rows, core-major,
             src_rows   [ROWS_TOTAL] staging row per lookup (same order),
             core_of    [ROWS_TOTAL] core id per lookup (same order)).
    """
    idx = np.asarray(indices)
    perm = np.asarray(PERM)
    glob = (idx[perm].astype(np.int64) + (perm * V)[:, None]).reshape(-1)

    core = glob // SHARD                       # [N]
    local = glob - core * SHARD
    win = local // WIN                         # window id 0..15
    wlocal = local - win * WIN                 # 0..32767

    group = core * N_WIN + win                 # 0..127
    order = np.lexsort((wlocal, group))        # grouped; row-sorted in group
    g_sorted = group[order]
    w_sorted = wlocal[order]

    # dedup: duplicates of a row within a bucket share one gather slot
    first = np.ones(glob.size, bool)
    first[1:] = (g_sorted[1:] != g_sorted[:-1]) | (w_sorted[1:] != w_sorted[:-1])
    fc = np.cumsum(first) - 1                  # global distinct ordinal
    gstart = np.ones(glob.size, bool)
    gstart[1:] = g_sorted[1:] != g_sorted[:-1]
    gs_idx = np.flatnonzero(gstart)
    run_len = np.diff(np.append(gs_idx, glob.size))
    slot = fc - np.repeat(fc[gs_idx], run_len)  # distinct slot within bucket

    n_groups = N_CORES * N_WIN
    dist_counts = np.zeros(n_groups, np.int64)
    dist_counts[g_sorted[gs_idx]] = (
        fc[np.append(gs_idx[1:] - 1, glob.size - 1)] - fc[gs_idx] + 1
    )
    pads = np.array(PADS * N_CORES)
    if np.any(dist_counts > pads):
        bad = np.argwhere(dist_counts > pads).ravel()
        raise RuntimeError(f"bucket overflow: {bad} {dist_counts[bad]}")

    # staging row: sub-gather s slot r lands at SBUF [p=r%128, col s*8+r//128]
    # of the window tile; the window writeback maps SBUF (p, c) to staging row
    # base_w + p*cols_w + c.
    stage_off = np.cumsum([0] + PADS)[:-1]     # per-window base within a core
    base = np.tile(stage_off, N_CORES)[g_sorted]
    cols = np.array(COLS * N_CORES)[g_sorted]
    sub, r = slot // GRANULE, slot % GRANULE
    src_rows = base + (r % P) * cols + sub * GCOLS + r // P

    # int16 index tiles, wrapped in 16 partitions: distinct slot d goes to
    # [partition d%16, col c0_w + d//16]; unused cells are -1 (skipped by the
    # runtime count register).
    idx_off16 = np.cumsum([0] + [p // 16 for p in PADS])[:-1]
    idx16 = np.zeros((N_CORES, 16, IDX_COLS), dtype=np.int16)
    c_of = g_sorted // N_WIN
    flat_cols = idx_off16[g_sorted % N_WIN] + slot // 16
    idx16[c_of, slot % 16, flat_cols] = w_sorted.astype(np.int16)

    # per-sub-gather valid counts; empty sub-gathers get one dummy (row 0)
    # because an all-negative gather is illegal.
    sub_of_pad = np.concatenate([[w] * (PADS[w] // GRANULE) for w in range(N_WIN)])
    sub_rank = np.concatenate(
        [np.arange(PADS[w] // GRANULE) for w in range(N_WIN)]
    )
    cnts = np.zeros((N_CORES, NSUB + N_WIN), np.int32)
    for c in range(N_CORES):
        wc = dist_counts[c * N_WIN : (c + 1) * N_WIN]
        sc = np.clip(wc[sub_of_pad] - sub_rank * GRANULE, 0, GRANULE)
        empty = sc == 0
        if np.any(empty):
            gi = np.flatnonzero(empty)
            cells = idx_off16[sub_of_pad[gi]] + sub_rank[gi] * (GRANULE // 16)
            idx16[c, 0, cells] = 0
            sc[gi] = 1
        cnts[c, :NSUB] = sc
        # writeback-B flag: last granule of each window written only when the
        # bucket spills past PADS[w] - GRANULE rows
        cnts[c, NSUB:] = (
            wc > (np.array(PADS) - GRANULE)
        ).astype(np.int32)

    idx_inputs = np.ascontiguousarray(np.tile(idx16, (1, 8, 1)))
    return idx_inputs, order, src_rows, c_of, cnts


_NC_CACHE = {}


def _get_nc():
    if "nc" not in _NC_CACHE:
        _NC_CACHE["nc"] = build_nc()
    return _NC_CACHE["nc"]


def run_sharded(indices, tables, trace=False, **spmd_kwargs):
    """Run the SPMD kernel on 8 cores; returns (full_output, BassKernelResults)."""
    from concourse import bass_utils

    tables_flat = np.asarray(tables, dtype=np.float32).reshape(F * V, D)
    idx_inputs, dst_rows, src_rows, core_of, cnts = route(indices)

    in_maps = []
    for c in range(N_CORES):
        m = {"idx": idx_inputs[c], "cnt": cnts[c : c + 1].reshape(1, NSUB + N_WIN)}
        shard = tables_flat[c * SHARD : (c + 1) * SHARD]
        r0 = 0
        for w in range(N_WIN):
            m[f"tab{w}"] = shard[r0 : r0 + WIN_ROWS[w]]
            r0 += WIN_ROWS[w]
        in_maps.append(m)

    nc = _get_nc()
    res = bass_utils.run_bass_kernel_spmd(
        nc, in_maps, list(range(N_CORES)), trace=trace, **spmd_kwargs
    )

    out_flat = np.empty((ROWS_TOTAL, D), dtype=np.float32)
    for c in range(N_CORES):
        sel = core_of == c
        out_flat[dst_rows[sel]] = res.results[c]["out"][src_rows[sel]]
    return out_flat.reshape(F, BL, D), res


def kernel(indices, tables):
    out, _ = run_sharded(indices, tables, trace=False)
    return out


# revision 27
# speedup vs baseline: 1.3815x; 1.3815x over previous
"""Trainium2 Bass kernel: multi-table embedding gather (pooling=NONE).

Reference computation (hardcoded shapes):
    indices: [F=4, BL=204800] int   (values in [0, V))
    tables:  [F=4, V=1e6, D=64] f32
    out[f]   = tables[PERM[f]][indices[PERM[f]]]   -> [4, 204800, 64] f32
    PERM = [2, 0, 3, 1]

Strategy (model/table-parallel, per the sharding hint):
  * Fold the table permutation into global row ids g = PERM[f]*V + idx over a
    flat [4M, 64] table.
  * Shard the flat table row-wise across the 8 cores (500,000 rows each).
    The host routes every lookup to the core owning its row, bucketing each
    core's lookups by 32,768-row window so the on-core gather can use the
    high-throughput int16 `dma_gather` SWDGE instruction (0.34 ns/descriptor
    vs ~1 us/instruction for generic indirect DMA, which HW only supports at
    128 rows per instruction).
  * Each core gathers its (padded) buckets window-by-window into SBUF and
    streams them to a contiguous staging output with large HWDGE DMAs.
  * The host applies the recorded inverse permutation to scatter staged rows
    into the final [4, 204800, 64] output (host-side unshard).
"""

import sys

import numpy as np

for _p in ("/opt/trn_rl_repo",):
    if _p not in sys.path:
        sys.path.insert(0, _p)

F = 4
V = 1_000_000
D = 64
BL = 204_800
PERM = (2, 0, 3, 1)

N_CORES = 8
P = 128
ROWS_TOTAL = F * BL                   # 819,200 lookups
SHARD = F * V // N_CORES              # 500,000 table rows per core
WIN = 32_768                          # int16-addressable window
N_FULL_WIN = SHARD // WIN             # 15 full windows
LAST_WIN_ROWS = SHARD - N_FULL_WIN * WIN  # 8,480
N_WIN = N_FULL_WIN + 1                # 16 windows per core

# Per-window bucket capacity (static padding; lookups are uniform so bucket
# sizes concentrate tightly: full-window mean 6711 sigma 82, last-window mean
# 1737 sigma 42).
PAD_FULL = 7_168                      # 56 * 128
PAD_LAST = 2_048                      # 16 * 128
PADS = [PAD_FULL] * N_FULL_WIN + [PAD_LAST]
COLS = [p // P for p in PADS]         # dst free-dim blocks per window
STAGE_ROWS = sum(PADS)                # 109,568 staged rows per core
IDX_COLS = sum(p // 16 for p in PADS)  # int16 idx columns: 6,848
WIN_ROWS = [WIN] * N_FULL_WIN + [LAST_WIN_ROWS]

NBUF = 7                # window dst tiles in flight (14KB/partition each)
GRANULE = 1024          # idxs per dma_gather (single-packet limit: 64 desc/engine)
GCOLS = GRANULE // P    # 8 dst free-dim blocks per sub-gather
N_SWDGE_QUEUES = 4
DMA_SCRATCH = 65536
NSUB = sum(p // GRANULE for p in PADS)   # 107 sub-gathers per core


def build_nc():
    """Per-core SPMD program: 16 dma_gather windows + contiguous writebacks."""
    import concourse.bacc as bacc
    import concourse.mybir as mybir
    import concourse.tile as tile

    nc = bacc.Bacc(
        None,
        num_swdge_queues=N_SWDGE_QUEUES,
        dynamic_dma_scratch_size=DMA_SCRATCH,
    )
    tabs = [
        nc.declare_dram_parameter(
            f"tab{w}", [WIN_ROWS[w], D], mybir.dt.float32, isOutput=False
        )
        for w in range(N_WIN)
    ]
    idx_in = nc.declare_dram_parameter(
        "idx", [P, IDX_COLS], mybir.dt.int16, isOutput=False
    )
    cnt_in = nc.declare_dram_parameter(
        "cnt", [1, NSUB + N_WIN], mybir.dt.int32, isOutput=False
    )
    out = nc.declare_dram_parameter(
        "out", [STAGE_ROWS, D], mybir.dt.float32, isOutput=True
    )

    idx_off = np.cumsum([0] + [p // 16 for p in PADS]).tolist()
    stage_off = np.cumsum([0] + PADS).tolist()

    g_idx = 0
    regs = [nc.alloc_register(mybir.EngineType.Pool, f"cnt_reg{i}") for i in range(24)]
    with tile.TileContext(nc) as tc:
        with (
            tc.tile_pool(name="idxp", bufs=1) as idxpool,
            tc.tile_pool(name="data", bufs=NBUF) as pool,
        ):
            idx_tile = idxpool.tile([P, IDX_COLS], mybir.dt.int16)
            cnt_tile = idxpool.tile([1, NSUB + N_WIN], mybir.dt.int32, tag="cnt")
            nc.sync.dma_start(out=cnt_tile[:], in_=cnt_in[:])
            # Tiny dummy gather up front so Bacc's auto library reload lands
            # at the top of the Pool stream, overlapping the preamble instead
            # of gating the first real gather.
            dummy_idx = idxpool.tile([P, 1], mybir.dt.int16, tag="dmyi")
            dummy_dst = idxpool.tile([P, 4 * D], mybir.dt.float32, tag="dmyd")
            nc.vector.memset(dummy_idx[:], 0)
            # one per queue: keeps Tile's DMASW lane<->queue phase aligned
            # for the real gather stream (lane k stays locked to queue k%4)
            for q in range(N_SWDGE_QUEUES):
                nc.gpsimd.dma_gather(
                    dummy_dst[:, q * D : (q + 1) * D].rearrange(
                        "p (c d) -> p c d", d=D
                    ),
                    tabs[0][:],
                    dummy_idx[:],
                    16,
                    16,
                    D,
                    single_packet=True,
                    queue_num=q,
                )
            # split the idx load per window so window 0 can start gathering
            # without waiting for the full 1.75MB index transfer
            for w in range(N_WIN):
                nc.sync.dma_start(
                    out=idx_tile[:, idx_off[w] : idx_off[w + 1]],
                    in_=idx_in[:, idx_off[w] : idx_off[w + 1]],
                )
            for w in range(N_WIN):
                # Split each window into 1024-idx single-packet gathers:
                # >64 descriptors/engine exceeds the packet limit (device
                # crash); four SWDGE queues keep ~4 packets in flight per
                # SDMA engine (latency-bound random 256B reads). All
                # sub-gathers of a window land in one tile (disjoint slices)
                # so the writeback is a single 1.8MB HWDGE DMA.
                data = pool.tile([P, COLS[w] * D], mybir.dt.float32, tag="data")
                for s in range(PADS[w] // GRANULE):
                    c0 = idx_off[w] + s * (GRANULE // 16)
                    f0 = s * GRANULE // P * D
                    # Runtime count register: the ucode only emits
                    # descriptors for the valid (deduped) prefix; trailing
                    # -1 idx slots are skipped.
                    reg = regs[g_idx % len(regs)]
                    nc.gpsimd.reg_load(reg, cnt_tile[0:1, g_idx : g_idx + 1])
                    nc.gpsimd.dma_gather(
                        data[:, f0 : f0 + GCOLS * D].rearrange(
                            "p (c d) -> p c d", d=D
                        ),
                        tabs[w][:],
                        idx_tile[:, c0 : c0 + GRANULE // 16],
                        GRANULE,
                        reg,
                        D,
                        single_packet=True,
                        queue_num=g_idx % N_SWDGE_QUEUES,
                    )
                    g_idx += 1
                win_ap = out[stage_off[w] : stage_off[w + 1], :].rearrange(
                    "(p c) d -> p (c d)", p=P
                )
                # ACT HWDGE ring: keeps writebacks off the SP ring that
                # carries the idx loads, and gives the write stream its own
                # packet slot in the SDMA round-robin.
                nc.scalar.dma_start(out=win_ap[:], in_=data[:])
    nc.compile()
    return nc


def route(indices):
    """Host-side routing: global ids -> per-core window buckets.

    Returns (idx_inputs [N_CORES, P, IDX_COLS] int16,
             dst_rows   [ROWS_TOTAL] original flat output rows, core-major,
             src_rows   [ROWS_TOTAL] staging row per lookup (same order),
             core_of    [ROWS_TOTAL] core id per lookup (same order)).
    """
    idx = np.asarray(indices)
    perm = np.asarray(PERM)
    glob = (idx[perm].astype(np.int64) + (perm * V)[:, None]).reshape(-1)

    core = glob // SHARD                       # [N]
    local = glob - core * SHARD
    win = local // WIN                         # window id 0..15
    wlocal = local - win * WIN                 # 0..32767

    group = core * N_WIN + win                 # 0..127
    order = np.lexsort((wlocal, group))        # grouped; row-sorted in group
    g_sorted = group[order]
    w_sorted = wlocal[order]

    # dedup: duplicates of a row within a bucket share one gather slot
    first = np.ones(glob.size, bool)
    first[1:] = (g_sorted[1:] != g_sorted[:-1]) | (w_sorted[1:] != w_sorted[:-1])
    fc = np.cumsum(first) - 1                  # global distinct ordinal
    gstart = np.ones(glob.size, bool)
    gstart[1:] = g_sorted[1:] != g_sorted[:-1]
    gs_idx = np.flatnonzero(gstart)
    run_len = np.diff(np.append(gs_idx, glob.size))
    slot = fc - np.repeat(fc[gs_idx], run_len)  # distinct slot within bucket

    n_groups = N_CORES * N_WIN
    dist_counts = np.zeros(n_groups, np.int64)
    dist_counts[g_sorted[gs_idx]] = (
        fc[np.append(gs_idx[1:] - 1, glob.size - 1)] - fc[gs_idx] + 1
    )
    pads = np.array(PADS * N_CORES)
    if np.any(dist_counts > pads):
        bad = np.argwhere(dist_counts > pads).ravel()
        raise RuntimeError(f"bucket overflow: {bad} {dist_counts[bad]}")

    # staging row: sub-gather s slot r lands at SBUF [p=r%128, col s*8+r//128]
    # of the window tile; the window writeback maps SBUF (p, c) to staging row
    # base_w + p*cols_w + c.
    stage_off = np.cumsum([0] + PADS)[:-1]     # per-window base within a core
    base = np.tile(stage_off, N_CORES)[g_sorted]
    cols = np.array(COLS * N_CORES)[g_sorted]
    sub, r = slot // GRANULE, slot % GRANULE
    src_rows = base + (r % P) * cols + sub * GCOLS + r // P

    # int16 index tiles, wrapped in 16 partitions: distinct slot d goes to
    # [partition d%16, col c0_w + d//16]; unused cells are -1 (skipped by the
    # runtime count register).
    idx_off16 = np.cumsum([0] + [p // 16 for p in PADS])[:-1]
    idx16 = np.full((N_CORES, 16, IDX_COLS), -1, dtype=np.int16)
    c_of = g_sorted // N_WIN
    flat_cols = idx_off16[g_sorted % N_WIN] + slot // 16
    idx16[c_of, slot % 16, flat_cols] = w_sorted.astype(np.int16)

    # per-sub-gather valid counts; empty sub-gathers get one dummy (row 0)
    # because an all-negative gather is illegal.
    sub_of_pad = np.concatenate([[w] * (PADS[w] // GRANULE) for w in range(N_WIN)])
    sub_rank = np.concatenate(
        [np.arange(PADS[w] // GRANULE) for w in range(N_WIN)]
    )
    cnts = np.zeros((N_CORES, NSUB + N_WIN), np.int32)
    for c in range(N_CORES):
        wc = dist_counts[c * N_WIN : (c + 1) * N_WIN]
        sc = np.clip(wc[sub_of_pad] - sub_rank * GRANULE, 0, GRANULE)
        empty = sc == 0
        if np.any(empty):
            gi = np.flatnonzero(empty)
            cells = idx_off16[sub_of_pad[gi]] + sub_rank[gi] * (GRANULE // 16)
            idx16[c, 0, cells] = 0
            sc[gi] = 1
        cnts[c, :NSUB] = sc
        # writeback-B flag: last granule of each window written only when the
        # bucket spills past PADS[w] - GRANULE rows
        cnts[c, NSUB:] = (
            wc > (np.array(PADS) - GRANULE)
        ).astype(np.int32)

    idx_inputs = np.ascontiguousarray(np.tile(idx16, (1, 8, 1)))
    return idx_inputs, order, src_rows, c_of, cnts


_NC_CACHE = {}


def _get_nc():
    if "nc" not in _NC_CACHE:
        _NC_CACHE["nc"] = build_nc()
    return _NC_CACHE["nc"]


def run_sharded(indices, tables, trace=False, **spmd_kwargs):
    """Run the SPMD kernel on 8 cores; returns (full_output, BassKernelResults)."""
    from concourse import bass_utils

    tables_flat = np.asarray(tables, dtype=np.float32).reshape(F * V, D)
    idx_inputs, dst_rows, src_rows, core_of, cnts = route(indices)

    in_maps = []
    for c in range(N_CORES):
        m = {"idx": idx_inputs[c], "cnt": cnts[c : c + 1].reshape(1, NSUB + N_WIN)}
        shard = tables_flat[c * SHARD : (c + 1) * SHARD]
        r0 = 0
        for w in range(N_WIN):
            m[f"tab{w}"] = shard[r0 : r0 + WIN_ROWS[w]]
            r0 += WIN_ROWS[w]
        in_maps.append(m)

    nc = _get_nc()
    res = bass_utils.run_bass_kernel_spmd(
        nc, in_maps, list(range(N_CORES)), trace=trace, **spmd_kwargs
    )

    out_flat = np.empty((ROWS_TOTAL, D), dtype=np.float32)
    for c in range(N_CORES):
        sel = core_of == c
        out_flat[dst_rows[sel]] = res.results[c]["out"][src_rows[sel]]
    return out_flat.reshape(F, BL, D), res


def kernel(indices, tables):
    out, _ = run_sharded(indices, tables, trace=False)
    return out
